# revision 1
# baseline (speedup 1.0000x reference)
"""FP8-palettized linear kernel for 8x TRN2 NeuronCores.

Computes: out[b,s,o] = sum_d input[b,s,d] * lookup_table[weight[o,d]] + bias[o]
with input [4,2048,4096] f32, weight [4096,4096] int32 (palette ids < 256),
lookup_table [256] f32, bias [4096] f32.

Strategy (column-parallel, per sharding hint):
  - Each core owns a 512-wide slice of out_features; input replicated.
  - Host prep is layout/dtype marshalling only: X is tiled into contiguous
    [128, 4096] X^T slabs (one 1MB DMA per m-tile instead of 4096 512B
    descriptors), palette indices are stored in the ap_gather wrapped order
    with the 256*(p%16) segment ramp pre-folded (an index-arithmetic
    relabeling), as int16.
  - On device, per k-tile: GPSIMD ap_gather reads a segment-expanded LUT
    (zeros outside the partition's own 256-slot window) producing
    red[p, o*16+r] = LUT[idx[o, d_p]] for r==p%16 else 0; a CONTIGUOUS
    inner-16 DVE tensor_reduce compacts it into a resident W^T tile
    [128 d, 512 o]. TensorE accumulates X^T-slab @ W^T over 32 k-tiles in
    PSUM; the first 8 m-tiles run k-outer across 8 PSUM banks so the PE
    rides the dequant wave instead of stalling on the last W^T tile.
    DVE adds bias, results DMA out per m-tile.
"""

import contextlib
import os

import ml_dtypes
import numpy as np

import concourse.bacc as bacc
import concourse.mybir as mybir
import concourse.tile as tile
from concourse import library_config
from concourse.bass_utils import run_bass_kernel_spmd

P = 128
N_CORES = 8

# Full-problem dims (hardcoded per harness contract).
BATCH, SEQ, D_IN, D_OUT, PALETTE = 4, 2048, 4096, 4096, 256
M_FULL = BATCH * SEQ  # 8192

MM_DTYPE = {
    "f32": mybir.dt.float32,
    "f32r": mybir.dt.float32r,
    "bf16": mybir.dt.bfloat16,
}[os.environ.get("PAL_MM_DTYPE", "bf16")]


def _np_mm_dtype():
    return (ml_dtypes.bfloat16 if MM_DTYPE == mybir.dt.bfloat16
            else np.float32)


def build_program(nc, *, m, k, osh, reps=1):
    """Emit the per-core Tile program. m: rows of X (mult of 128), k: d dim
    (mult of 128), osh: out-features per core (512). reps>1 wraps the body
    in a hardware loop (benchmarking: amortizes dispatch overhead)."""
    n_kt = k // P
    n_mt = m // P
    f_red = 16 * osh  # gather output free size (per-partition)
    # Phase-1 depth: m-tiles accumulated k-outer across PSUM banks while
    # dequant streams in. 8 banks single-shot; 4 when the rep loop is
    # unrolled 2x (SBUF holds two W^T sets for cross-rep overlap).
    unroll = reps > 1
    np1 = (4 if unroll else 8) if MM_DTYPE == mybir.dt.bfloat16 else 2

    xt = nc.dram_tensor("xt", [m, k], MM_DTYPE, kind="ExternalInput")
    widx = nc.dram_tensor("widx", [n_kt * P, osh], mybir.dt.int16,
                          kind="ExternalInput")
    lutx = nc.dram_tensor("lutx", [P, 16 * PALETTE], mybir.dt.float32,
                          kind="ExternalInput")
    bias = nc.dram_tensor("bias", [P, osh], mybir.dt.float32,
                          kind="ExternalInput")
    out = nc.dram_tensor("out", [m, osh], mybir.dt.float32,
                         kind="ExternalOutput")
    # K-split spill buffer (single-shot path): phase-A partials live here.
    scr = nc.dram_tensor("scr", [m, osh], mybir.dt.float32,
                         kind="ExternalOutput")

    with tile.TileContext(nc) as tc:
        with (
            tc.tile_pool(name="const", bufs=1) as const_pool,
            tc.tile_pool(name="idx", bufs=2) as idx_pool,
            tc.tile_pool(name="red", bufs=2) as red_pool,
            tc.tile_pool(name="wt", bufs=1) as wt_pool,
            tc.tile_pool(name="xs", bufs=1) as x_pool,
            tc.tile_pool(name="psum", bufs=1, space="PSUM") as psum_pool,
            tc.tile_pool(name="osb", bufs=3) as osb_pool,
        ):
            # Pin the gather library + constants once per dispatch; the
            # rep loop then starts its gathers with no GPSIMD drain.
            nc.gpsimd.load_library(library_config.ap_gather)
            lutx_sb = const_pool.tile([P, 16 * PALETTE], mybir.dt.float32,
                                      tag="lutx")
            nc.sync.dma_start(lutx_sb[:], lutx[:])
            bias_sb = const_pool.tile([P, osh], mybir.dt.float32, tag="bsb")
            nc.sync.dma_start(bias_sb[:], bias[:])

            def rep_body(par):
                # --- dequant: W^T tile [128 d, osh o] per k-tile ---
                # wt is parity-tagged so this wave overlaps the other
                # parity's matmul tail; idx/red rotate via pool bufs.
                wt_tiles = []
                for kt in range(n_kt):
                    idxt = idx_pool.tile([P, osh], mybir.dt.int16,
                                         tag="idx", name=f"idx{par}_{kt}")
                    nc.scalar.dma_start(idxt[:],
                                        widx[kt * P:(kt + 1) * P, :])
                    red = red_pool.tile([P, f_red], mybir.dt.float32,
                                        tag="red", name=f"red{par}_{kt}")
                    nc.gpsimd.ap_gather(
                        red[:], lutx_sb[:], idxt[:],
                        channels=P, num_elems=16 * PALETTE, d=1,
                        num_idxs=f_red)
                    wt = wt_pool.tile([P, osh], MM_DTYPE,
                                      tag=f"wt{par}{kt:02d}",
                                      name=f"wt{par}_{kt}")
                    # exact: each 16-group is 15 zeros + the wanted value
                    with nc.allow_low_precision(
                            reason="sum of one value + zeros"):
                        nc.vector.tensor_reduce(
                            wt[:],
                            red.rearrange("p (o r) -> p o r", r=16),
                            axis=mybir.AxisListType.X,
                            op=mybir.AluOpType.add)
                    wt_tiles.append(wt)

                def load_xslab(mt, slot):
                    xslab = x_pool.tile([P, k], MM_DTYPE, tag=f"xs{slot}",
                                        name=f"xs{par}_{mt}")
                    nc.sync.dma_start(xslab[:], xt[mt * P:(mt + 1) * P, :])
                    return xslab

                def finish_mtile(mt, psum):
                    osb = osb_pool.tile([P, osh], mybir.dt.float32,
                                        tag="osb", name=f"osb{par}_{mt}")
                    nc.vector.tensor_tensor(
                        osb[:], psum[:], bias_sb[:], op=mybir.AluOpType.add)
                    nc.scalar.dma_start(out[mt * P:(mt + 1) * P, :], osb[:])

                if not unroll:
                    # Single-shot: split K in half so ALL 64 m-tiles run
                    # their first-half accumulation during the gather wave
                    # (spilling fp32 partials+bias to HBM); only the
                    # second half remains after the last W^T tile.
                    half = n_kt // 2

                    def half_pass(mt, kts, xoff, bank):
                        xs = x_pool.tile([P, half * P], MM_DTYPE,
                                         tag=f"xh{bank}",
                                         name=f"x{kts[0]}_{mt}")
                        nc.sync.dma_start(
                            xs[:], xt[mt * P:(mt + 1) * P,
                                      xoff:xoff + half * P])
                        ps = psum_pool.tile([P, osh], mybir.dt.float32,
                                            tag=f"ps{bank}",
                                            name=f"ps{kts[0]}_{mt}")
                        for i, kt in enumerate(kts):
                            nc.tensor.matmul(
                                ps[:],
                                lhsT=xs[:, i * P:(i + 1) * P],
                                rhs=wt_tiles[kt][:],
                                start=(i == 0),
                                stop=(i == len(kts) - 1))
                        return ps

                    for mt in range(n_mt):
                        ps = half_pass(mt, range(half), 0, mt % 4)
                        osa = osb_pool.tile([P, osh], mybir.dt.float32,
                                            tag="osb", name=f"oa_{mt}")
                        nc.vector.tensor_tensor(
                            osa[:], ps[:], bias_sb[:],
                            op=mybir.AluOpType.add)
                        nc.scalar.dma_start(
                            scr[mt * P:(mt + 1) * P, :], osa[:])
                    for mt in range(n_mt):
                        ps = half_pass(mt, range(half, n_kt),
                                       half * P, 4 + mt % 4)
                        ora = osb_pool.tile([P, osh], mybir.dt.float32,
                                            tag="osb", name=f"or_{mt}")
                        nc.sync.dma_start(ora[:],
                                          scr[mt * P:(mt + 1) * P, :])
                        osb = osb_pool.tile([P, osh], mybir.dt.float32,
                                            tag="osb", name=f"ob_{mt}")
                        nc.vector.tensor_tensor(
                            osb[:], ps[:], ora[:], op=mybir.AluOpType.add)
                        nc.scalar.dma_start(
                            out[mt * P:(mt + 1) * P, :], osb[:])
                    return

                # --- phase 1: np1 m-tiles k-outer across PSUM banks ---
                slabs = [load_xslab(mt, mt % np1) for mt in range(np1)]
                psums = [psum_pool.tile([P, osh], mybir.dt.float32,
                                        tag=f"ps{i}", name=f"ps{par}_{i}")
                         for i in range(np1)]
                for kt in range(n_kt):
                    for i in range(np1):
                        nc.tensor.matmul(
                            psums[i][:],
                            lhsT=slabs[i][:, kt * P:(kt + 1) * P],
                            rhs=wt_tiles[kt][:],
                            start=(kt == 0),
                            stop=(kt == n_kt - 1))
                for i in range(np1):
                    finish_mtile(i, psums[i])

                # --- phase 2: remaining m-tiles, m-outer ---
                for mt in range(np1, n_mt):
                    xslab = load_xslab(mt, mt % np1)
                    psum = psum_pool.tile([P, osh], mybir.dt.float32,
                                          tag=f"ps{mt % np1}",
                                          name=f"ps{par}_{mt}")
                    for kt in range(n_kt):
                        nc.tensor.matmul(
                            psum[:],
                            lhsT=xslab[:, kt * P:(kt + 1) * P],
                            rhs=wt_tiles[kt][:],
                            start=(kt == 0),
                            stop=(kt == n_kt - 1))
                    finish_mtile(mt, psum)

            if unroll:
                assert reps % 2 == 0, "reps must be even for 2x unroll"
                with tc.For_i(0, reps // 2, 1):
                    rep_body(0)
                    rep_body(1)
            else:
                rep_body(0)

    return xt, widx, lutx, bias, out


def make_core_inputs(input, lookup_table, weight, bias, *, m=M_FULL, k=D_IN,
                     osh=D_OUT // N_CORES, n_cores=N_CORES):
    """Host-side sharding/layout prep (no palette lookups). Returns in_maps."""
    n_kt = k // P
    n_mt = m // P
    x2 = np.asarray(input, dtype=np.float32).reshape(m, k)
    # xt[mt, p, kt*128+j] = X[mt*128+j, kt*128+p]
    xt = (x2.reshape(n_mt, P, n_kt, P).transpose(0, 3, 2, 1)
          .reshape(m, k).astype(_np_mm_dtype()))

    lut_vals = np.asarray(lookup_table, dtype=np.float32).reshape(PALETTE)
    lutx = np.zeros((P, 16 * PALETTE), dtype=np.float32)
    for p in range(P):
        s = p % 16
        lutx[p, s * PALETTE:(s + 1) * PALETTE] = lut_vals

    weight = np.asarray(weight)
    bias = np.asarray(bias, dtype=np.float32)
    ramp = (PALETTE * (np.arange(P) % 16)).astype(np.int32)[None, :, None]

    in_maps = []
    for c in range(n_cores):
        w_shard = weight[c * osh:(c + 1) * osh, :]  # [osh, k] int32
        # widx[kt, p, s] = idx[s, kt*128+p] + 256*(p%16)
        widx = (w_shard.T.reshape(n_kt, P, osh) + ramp).astype(np.int16)
        in_maps.append({
            "xt": xt,
            "widx": widx.reshape(n_kt * P, osh),
            "lutx": lutx,
            "bias": np.broadcast_to(bias[c * osh:(c + 1) * osh],
                                    (P, osh)).copy(),
        })
    return in_maps


def kernel(input, lookup_table, weight, bias, *, trace=False):
    osh = D_OUT // N_CORES
    nc = bacc.Bacc("TRN2", target_bir_lowering=False, debug=False,
                   num_devices=N_CORES)
    build_program(nc, m=M_FULL, k=D_IN, osh=osh)
    nc.compile()

    in_maps = make_core_inputs(input, lookup_table, weight, bias)
    res = run_bass_kernel_spmd(nc, in_maps, core_ids=list(range(N_CORES)),
                               trace=trace)
    out = np.concatenate([r["out"] for r in res.results], axis=1)
    out = np.ascontiguousarray(out.reshape(BATCH, SEQ, D_OUT), dtype=np.float32)
    if trace:
        kernel.last_results = res
    return out



# revision 2
# speedup vs baseline: 19.4515x; 19.4515x over previous
"""FP8-palettized linear kernel for 8x TRN2 NeuronCores.

Computes: out[b,s,o] = sum_d input[b,s,d] * lookup_table[weight[o,d]] + bias[o]
with input [4,2048,4096] f32, weight [4096,4096] int32 (palette ids < 256),
lookup_table [256] f32, bias [4096] f32.

Strategy (column-parallel, per sharding hint):
  - Each core owns a 512-wide slice of out_features; input replicated.
  - Dequantization runs on the ScalarE (ACT) spline evaluator: at
    kernel() time we bake the 256-entry palette into a custom PWP
    activation table (a piecewise-constant staircase f(x) = LUT[round(x)]
    hijacking the 'gelu' slot, handed to walrus via
    BASS_ACT_ROOT_JSON_PATH — see act_table.py). Palette indices are
    shipped as bf16 (exact for 0..255); one ACTIVATE per k-tile turns
    the [128, 512] index tile into the bf16 W^T tile at 1 elem/cycle/lane
    (~720ns/tile), bit-identical to gather+round dequant.
  - Host prep is layout/dtype marshalling only: X tiled into contiguous
    [128, 4096] X^T slabs (one 1MB DMA per m-tile), weight indices
    transposed into k-tile-major [128, osh] tiles as bf16.
  - TensorE accumulates X^T-slab @ W^T over 32 k-tiles in PSUM; the first
    8 m-tiles run k-outer across the 8 PSUM banks so the PE starts while
    dequant + X DMA are still streaming; remaining m-tiles run m-outer
    with rotating banks. DVE adds bias, results DMA out per m-tile.
  - The LUT content is folded into the widx tensor name so the NEFF cache
    key changes whenever the activation table must change.
"""

import hashlib
import os
import tempfile

import ml_dtypes
import numpy as np

import concourse.bacc as bacc
import concourse.mybir as mybir
import concourse.tile as tile
from concourse.bass_utils import run_bass_kernel_spmd

import act_table

P = 128
N_CORES = 8

# Full-problem dims (hardcoded per harness contract).
BATCH, SEQ, D_IN, D_OUT, PALETTE = 4, 2048, 4096, 4096, 256
M_FULL = BATCH * SEQ  # 8192

MM_DTYPE = mybir.dt.bfloat16


def _np_mm_dtype():
    return ml_dtypes.bfloat16


def lut_tag(lookup_table):
    lut = np.asarray(lookup_table, dtype=np.float32).reshape(PALETTE)
    return hashlib.sha1(lut.tobytes()).hexdigest()[:12]


def install_act_tables(lookup_table):
    """Bake the palette into a custom ACT table root and point walrus at
    it. Must run before the NEFF compile."""
    lut = np.asarray(lookup_table, dtype=np.float32).reshape(PALETTE)
    root = act_table.build_act_root(
        lut, tempfile.mkdtemp(prefix=f"actroot_{lut_tag(lut)}_"))
    os.environ["BASS_ACT_ROOT_JSON_PATH"] = root
    return root


def build_program(nc, *, m, k, osh, ltag, reps=1):
    """Emit the per-core Tile program. m: rows of X (mult of 128), k: d dim
    (mult of 128), osh: out-features per core (512). reps>1 wraps the body
    in a hardware loop (benchmarking: amortizes dispatch overhead).
    ltag: hash of the lookup table (cache-keys the NEFF to the act root)."""
    n_kt = k // P
    n_mt = m // P
    np1 = 8  # phase-1 m-tiles, one per PSUM bank

    xt = nc.dram_tensor("xt", [m, k], MM_DTYPE, kind="ExternalInput")
    widx = nc.dram_tensor(f"widx_{ltag}", [n_kt * P, osh], MM_DTYPE,
                          kind="ExternalInput")
    bias = nc.dram_tensor("bias", [P, osh], mybir.dt.float32,
                          kind="ExternalInput")
    out = nc.dram_tensor("out", [m, osh], mybir.dt.float32,
                         kind="ExternalOutput")

    with tile.TileContext(nc) as tc:
        with (
            tc.tile_pool(name="const", bufs=1) as const_pool,
            tc.tile_pool(name="idx", bufs=2) as idx_pool,
            tc.tile_pool(name="wt", bufs=2) as wt_pool,
            tc.tile_pool(name="xs", bufs=2) as x_pool,
            tc.tile_pool(name="psum", bufs=1, space="PSUM") as psum_pool,
            tc.tile_pool(name="osb", bufs=3) as osb_pool,
        ):
            bias_sb = const_pool.tile([P, osh], mybir.dt.float32, tag="bsb")
            nc.sync.dma_start(bias_sb[:], bias[:])

            def rep_body():
                # --- dequant: ACT staircase turns idx tiles into W^T ---
                wt_tiles = []
                for kt in range(n_kt):
                    idxt = idx_pool.tile([P, osh], MM_DTYPE,
                                         tag="idx", name=f"idx{kt}")
                    nc.scalar.dma_start(idxt[:],
                                        widx[kt * P:(kt + 1) * P, :])
                    wt = wt_pool.tile([P, osh], MM_DTYPE,
                                      tag=f"wt{kt:02d}", name=f"wt{kt}")
                    nc.scalar.activation(
                        wt[:], idxt[:], mybir.ActivationFunctionType.Gelu)
                    wt_tiles.append(wt)

                def load_xslab(mt):
                    xslab = x_pool.tile([P, k], MM_DTYPE,
                                        tag=f"xs{mt % np1}", name=f"xs{mt}")
                    nc.sync.dma_start(xslab[:], xt[mt * P:(mt + 1) * P, :])
                    return xslab

                def finish_mtile(mt, psum):
                    osb = osb_pool.tile([P, osh], mybir.dt.float32,
                                        tag="osb", name=f"osb{mt}")
                    nc.vector.tensor_tensor(
                        osb[:], psum[:], bias_sb[:], op=mybir.AluOpType.add)
                    nc.scalar.dma_start(out[mt * P:(mt + 1) * P, :], osb[:])

                # --- phase 1: np1 m-tiles k-outer across PSUM banks ---
                slabs = [load_xslab(mt) for mt in range(np1)]
                psums = [psum_pool.tile([P, osh], mybir.dt.float32,
                                        tag=f"ps{i}", name=f"ps{i}")
                         for i in range(np1)]
                for kt in range(n_kt):
                    for i in range(np1):
                        nc.tensor.matmul(
                            psums[i][:],
                            lhsT=slabs[i][:, kt * P:(kt + 1) * P],
                            rhs=wt_tiles[kt][:],
                            start=(kt == 0),
                            stop=(kt == n_kt - 1))
                for i in range(np1):
                    finish_mtile(i, psums[i])

                # --- phase 2: remaining m-tiles, m-outer ---
                for mt in range(np1, n_mt):
                    xslab = load_xslab(mt)
                    psum = psum_pool.tile([P, osh], mybir.dt.float32,
                                          tag=f"ps{mt % np1}",
                                          name=f"ps{mt}")
                    for kt in range(n_kt):
                        nc.tensor.matmul(
                            psum[:],
                            lhsT=xslab[:, kt * P:(kt + 1) * P],
                            rhs=wt_tiles[kt][:],
                            start=(kt == 0),
                            stop=(kt == n_kt - 1))
                    finish_mtile(mt, psum)

            if reps > 1:
                with tc.For_i(0, reps, 1):
                    rep_body()
            else:
                rep_body()

    return xt, widx, bias, out


def make_core_inputs(input, lookup_table, weight, bias, *, m=M_FULL, k=D_IN,
                     osh=D_OUT // N_CORES, n_cores=N_CORES):
    """Host-side sharding/layout prep (no palette lookups). Returns in_maps.
    Also installs the act-table root for the palette (env for the compile)."""
    install_act_tables(lookup_table)
    ltag = lut_tag(lookup_table)

    n_kt = k // P
    n_mt = m // P
    x2 = np.asarray(input, dtype=np.float32).reshape(m, k)
    # xt[mt, p, kt*128+j] = X[mt*128+j, kt*128+p]
    xt = (x2.reshape(n_mt, P, n_kt, P).transpose(0, 3, 2, 1)
          .reshape(m, k).astype(_np_mm_dtype()))

    weight = np.asarray(weight)
    bias = np.asarray(bias, dtype=np.float32)

    in_maps = []
    for c in range(n_cores):
        w_shard = weight[c * osh:(c + 1) * osh, :]  # [osh, k] int32
        # widx[kt*128+p, o] = weight[c*osh+o, kt*128+p] as bf16 (ids < 256
        # are exact in bf16)
        widx = w_shard.T.astype(_np_mm_dtype())
        in_maps.append({
            "xt": xt,
            f"widx_{ltag}": np.ascontiguousarray(widx),
            "bias": np.broadcast_to(bias[c * osh:(c + 1) * osh],
                                    (P, osh)).copy(),
        })
    return in_maps


def kernel(input, lookup_table, weight, bias, *, trace=False):
    osh = D_OUT // N_CORES
    in_maps = make_core_inputs(input, lookup_table, weight, bias)

    nc = bacc.Bacc("TRN2", target_bir_lowering=False, debug=False,
                   num_devices=N_CORES)
    build_program(nc, m=M_FULL, k=D_IN, osh=osh, ltag=lut_tag(lookup_table))
    nc.compile()

    res = run_bass_kernel_spmd(nc, in_maps, core_ids=list(range(N_CORES)),
                               trace=trace)
    out = np.concatenate([r["out"] for r in res.results], axis=1)
    out = np.ascontiguousarray(out.reshape(BATCH, SEQ, D_OUT),
                               dtype=np.float32)
    if trace:
        kernel.last_results = res
    return out
